# revision 19
# baseline (speedup 1.0000x reference)
"""Local-strided block-sparse paged attention (decode) on 8 Trainium2 cores.

Strategy (memory-bound -> minimize device HBM bytes and DMA/descriptor serialization):
- Host resolves the per-(batch, q-head) CSR rows, then DEDUPLICATES the kv
  blocks across the 4 q-heads of each kv-head group: one gathered K/V panel
  per (b, kv_head) union, with per-head multiplicative masks restoring each
  head's exact row + causal masking.
- Panels are fp16 (halves bytes; ~5e-4 rel err, tolerance 2e-2).
- The 64 panels are assigned to 8 cores x 8 slots sorted by size, so the
  SPMD slot padding (same program on all cores) wastes little bandwidth.
- Two DMAs per slot: [K|mask] issued by the Sync engine, [V] by the Scalar
  engine -- descriptor generation (~630ns/DMA) runs on both engines in
  parallel instead of serializing on Sync.
- Device per slot (all 4 heads batched per matmul):
    QK:   per 128-token chunk: stationary K-chunk [128d,128t] (FWL),
          moving q [128,4] -> scores[t, 4h] in PSUM
    p   = exp(scores*scale) (ACT, fp16 out) * mask (DVE fp16)
    PV:   per chunk: stationary V-chunk [128t,128d] (FWL), moving p-chunk
          [128,4] -> accumulates out [128d, 4h] in PSUM
    den:  ones[128,1]^T @ p -> per-(chunk,head) sums; host reduces + divides
- Outputs batched into 2 final DMAs; host does the final normalization.
"""
import numpy as np

B, H, KVH, D, X = 16, 16, 4, 128, 4
GRP = H // KVH              # q heads per kv head
BLK, MAXB = 16, 256
NC_CORES = 8
NSLOTS = (B * KVH) // NC_CORES   # panels per core
SM_SCALE = 1.0 / float(np.sqrt(D))

_PROG_CACHE = {}


def _build_device_program(slot_nch):
    import concourse.bacc as bacc
    import concourse.mybir as mybir
    from concourse.tile import TileContext

    f32 = mybir.dt.float32
    f16 = mybir.dt.float16
    nc = bacc.Bacc("TRN2", target_bir_lowering=False)
    # per-slot [K | mask] and [V] panels
    km = [nc.dram_tensor(f"km{s}", [128, slot_nch[s] * 132], f16, kind="ExternalInput")
          for s in range(NSLOTS)]
    vv = [nc.dram_tensor(f"vv{s}", [128, slot_nch[s] * 128], f16, kind="ExternalInput")
          for s in range(NSLOTS)]
    qd = nc.dram_tensor("qd", [128, 4 * NSLOTS], f16, kind="ExternalInput")
    oud = nc.dram_tensor("oud", [128, 4 * NSLOTS], f32, kind="ExternalOutput")
    dend = nc.dram_tensor("dend", [1, 4 * NSLOTS], f32, kind="ExternalOutput")

    with TileContext(nc) as tc:
        with (
            tc.tile_pool(name="data", bufs=1) as dp,
            tc.tile_pool(name="work", bufs=3) as wp,
            tc.tile_pool(name="ps_sc", bufs=2, space="PSUM") as psc,
            tc.tile_pool(name="ps_ov", bufs=2, space="PSUM") as pov,
        ):
            qt = dp.tile([128, 4 * NSLOTS], f16, tag="q")
            nc.sync.dma_start(out=qt[:], in_=qd[:])
            osb = dp.tile([128, 4 * NSLOTS], f32, tag="osb")
            dsb = dp.tile([1, 4 * NSLOTS], f32, tag="dsb")

            # K pieces as separate tiles: chunk boundaries per slot so QK(s)
            # can start on a piece while the rest of the slot still streams.
            def piece_cuts(s, nch):
                if s == 0:
                    cuts = [0, 4, (nch + 4) // 2, nch]
                elif s in (1, 2):
                    cuts = [0, (nch + 1) // 2, nch]
                else:
                    cuts = [0, nch]
                return sorted(set(c for c in cuts if 0 <= c <= nch))

            kp = [None] * NSLOTS   # list of (tile, c0, c1) pieces; mask in last
            vvs = [None] * NSLOTS

            def issue_km(s):
                nch = slot_nch[s]
                cuts = piece_cuts(s, nch)
                pieces = []
                for i, (c0, c1) in enumerate(zip(cuts[:-1], cuts[1:])):
                    last = i == len(cuts) - 2
                    w = (c1 - c0) * 128 + (nch * 4 if last else 0)
                    t = dp.tile([128, w], f16, tag=f"km{s}_{i}")
                    nc.sync.dma_start(
                        out=t[:], in_=km[s][:, c0 * 128:c0 * 128 + w])
                    pieces.append((t, c0, c1))
                kp[s] = pieces

            def issue_vv(s):
                nch = slot_nch[s]
                vvt = dp.tile([128, nch * 128], f16, tag=f"vv{s}")
                nc.sync.dma_start(out=vvt[:], in_=vv[s][:])
                vvs[s] = vvt

            # stream order: K runs one slot ahead of V so PE's QK is never
            # the last thing waiting on data
            issue_km(0)
            issue_km(1)
            issue_vv(0)
            for s in range(2, NSLOTS):
                issue_km(s)
                issue_vv(s - 1)
            issue_vv(NSLOTS - 1)

            ps = [None] * NSLOTS

            def qk_softmax(s):
                nch = slot_nch[s]
                sc = psc.tile([128, nch * 4], f32, tag="sc")
                for t, c0, c1 in kp[s]:
                    for c in range(c0, c1):
                        nc.tensor.matmul(
                            sc[:, 4 * c:4 * c + 4],
                            t[:, 128 * (c - c0):128 * (c - c0 + 1)],
                            qt[:, 4 * s:4 * s + 4],
                            start=True, stop=True,
                        )
                p0 = wp.tile([128, nch * 4], f16, tag="p0")
                nc.scalar.activation(
                    p0[:], sc[:], mybir.ActivationFunctionType.Exp,
                    scale=SM_SCALE,
                )
                lt, lc0, lc1 = kp[s][-1]
                moff = 128 * (lc1 - lc0)
                p = wp.tile([128, nch * 4], f16, tag="p")
                nc.vector.tensor_mul(
                    p[:], p0[:], lt[:, moff:moff + nch * 4])
                ps[s] = p

            def pv(s):
                nch = slot_nch[s]
                ov = pov.tile([128, 4], f32, tag="ov")
                for c in range(nch):
                    nc.tensor.matmul(
                        ov[:],
                        vvs[s][:, 128 * c:128 * (c + 1)],
                        ps[s][:, 4 * c:4 * c + 4],
                        start=(c == 0), stop=(c == nch - 1),
                    )
                ds = wp.tile([1, nch * 4], f32, tag="ds")
                nc.gpsimd.tensor_reduce(
                    ds[:], ps[s][:], mybir.AxisListType.C, mybir.AluOpType.add)
                nc.vector.tensor_copy(osb[:, 4 * s:4 * s + 4], ov[:])
                nc.vector.tensor_reduce(
                    dsb[:, 4 * s:4 * s + 4],
                    ds[:].rearrange("p (c h) -> p h c", h=4),
                    mybir.AxisListType.X, mybir.AluOpType.add)
                ps[s] = None

            # software pipeline: PE does QK(s+1) while ACT/DVE produce p(s);
            # PE never waits on the softmax chain.
            qk_softmax(0)
            for s in range(1, NSLOTS):
                qk_softmax(s)
                pv(s - 1)
            pv(NSLOTS - 1)

            nc.sync.dma_start(out=oud[:], in_=osb[:])
            nc.scalar.dma_start(out=dend[:], in_=dsb[:])
    nc.compile()
    return nc


def _prep(q, k_cache, v_cache, block_tables, context_lens, layout_crow, layout_col):
    """Resolve CSR rows, dedup kv blocks per (b, kv-head), build panels."""
    q_pid = context_lens.astype(np.int64) - 1            # [B]
    pbid = q_pid // BLK
    h_idx = np.arange(H)
    start = layout_crow[h_idx[None, :], pbid[:, None]]   # [B,H]
    end = layout_crow[h_idx[None, :], pbid[:, None] + 1]

    panels = []  # (nch, b, kv, U, cols_per_head)
    for b in range(B):
        for kv in range(KVH):
            cols_h = []
            for dh in range(GRP):
                h = kv * GRP + dh
                cols_h.append(layout_col[h, start[b, h]:end[b, h]])
            U = np.unique(np.concatenate(cols_h))
            nch = max(1, -(-(len(U) * BLK) // 128))
            panels.append((nch, b, kv, U, cols_h))

    order = sorted(range(len(panels)), key=lambda i: -panels[i][0])
    # slot processing order: smallest group first (compute starts after a
    # small DMA), then descending, second-smallest last (short pipeline drain)
    group_to_pos = list(range(NSLOTS))   # size-rank group -> slot position (descending)
    slot_nch = [0] * NSLOTS
    assign = [[None] * NSLOTS for _ in range(NC_CORES)]
    for rank, pi in enumerate(order):
        core, g = rank % NC_CORES, rank // NC_CORES
        pos = group_to_pos[g]
        assign[core][pos] = pi
        if core == 0:
            slot_nch[pos] = panels[pi][0]
    slot_nch = tuple(slot_nch)

    in_maps = []
    meta = []    # per core: list of (b, kv) per slot
    tok16 = np.arange(BLK)
    for core in range(NC_CORES):
        im = {}
        mt_core = []
        qd = np.zeros((128, 4 * NSLOTS), np.float16)
        for s in range(NSLOTS):
            nch, b, kv, U, cols_h = panels[assign[core][s]]
            NT = slot_nch[s] * 128
            NU = len(U)
            phys = block_tables[b, U]

            kmt = np.zeros((128, slot_nch[s] * 132), np.float16)
            kb = k_cache[phys, kv]                       # [NU, 32, 16, 4]
            kmt[:, :NU * BLK] = kb.transpose(1, 3, 0, 2).reshape(128, NU * BLK)

            vb = v_cache[phys, kv]                       # [NU, 128, 16]
            v_t = np.zeros((NT, 128), np.float16)
            v_t[:NU * BLK] = vb.transpose(0, 2, 1).reshape(NU * BLK, 128)
            vvt = np.ascontiguousarray(
                v_t.reshape(slot_nch[s], 128, 128).transpose(1, 0, 2)
                .reshape(128, NT))

            mm = np.zeros((4, NT), np.float16)
            upos = U * BLK
            causal = (upos[:, None] + tok16[None, :]) <= q_pid[b]   # [NU,16]
            for dh in range(GRP):
                allowed = np.isin(U, cols_h[dh])[:, None] & causal
                mm[dh, :NU * BLK] = allowed.reshape(-1).astype(np.float16)
            kmt[:, NT:] = (
                mm.reshape(4, slot_nch[s], 128).transpose(2, 1, 0)
                .reshape(128, slot_nch[s] * 4))

            im[f"km{s}"] = kmt
            im[f"vv{s}"] = vvt
            qd[:, 4 * s:4 * s + 4] = q[b, kv * GRP:(kv + 1) * GRP].T
            mt_core.append((b, kv))
        im["qd"] = qd
        in_maps.append(im)
        meta.append(mt_core)
    return slot_nch, in_maps, meta


def kernel(q, k_cache, v_cache, block_tables, context_lens, layout_crow, layout_col):
    import os
    from concourse.bass_utils import run_bass_kernel_spmd

    q = np.asarray(q, np.float32)
    k_cache = np.asarray(k_cache, np.float32)
    v_cache = np.asarray(v_cache, np.float32)
    block_tables = np.asarray(block_tables, np.int32)
    context_lens = np.asarray(context_lens, np.int32)
    layout_crow = np.asarray(layout_crow, np.int32)
    layout_col = np.asarray(layout_col, np.int32)

    slot_nch, in_maps, meta = _prep(
        q, k_cache, v_cache, block_tables, context_lens, layout_crow, layout_col)

    nc = _PROG_CACHE.get(slot_nch)
    if nc is None:
        nc = _build_device_program(slot_nch)
        _PROG_CACHE[slot_nch] = nc

    res = run_bass_kernel_spmd(
        nc, in_maps, core_ids=list(range(NC_CORES)),
        trace=bool(os.environ.get("KERNEL_TRACE")),
    )
    global _LAST_RESULT
    _LAST_RESULT = res

    out = np.empty((B, H, D), np.float32)
    for core in range(NC_CORES):
        oud = res.results[core]["oud"]                   # [128, 4*NSLOTS]
        den = res.results[core]["dend"][0]               # [4*NSLOTS]
        for s in range(NSLOTS):
            b, kv = meta[core][s]
            out[b, kv * GRP:(kv + 1) * GRP] = (
                oud[:, 4 * s:4 * s + 4] / den[4 * s:4 * s + 4]).T
    return out


_LAST_RESULT = None


# revision 22
# speedup vs baseline: 1.8140x; 1.8140x over previous
"""Local-strided block-sparse paged attention (decode) on 8 Trainium2 cores.

Strategy (memory-bound -> minimize device HBM bytes and DMA/descriptor serialization):
- Host resolves the per-(batch, q-head) CSR rows, then DEDUPLICATES the kv
  blocks across the 4 q-heads of each kv-head group: one gathered K/V panel
  per (b, kv_head) union, with per-head multiplicative masks restoring each
  head's exact row + causal masking.
- Panels are fp16 (halves bytes; ~5e-4 rel err, tolerance 2e-2).
- The 64 panels are assigned to 8 cores x 8 slots sorted by size, so the
  SPMD slot padding (same program on all cores) wastes little bandwidth.
- Two DMAs per slot: [K|mask] issued by the Sync engine, [V] by the Scalar
  engine -- descriptor generation (~630ns/DMA) runs on both engines in
  parallel instead of serializing on Sync.
- Device per slot (all 4 heads batched per matmul):
    QK:   per 128-token chunk: stationary K-chunk [128d,128t] (FWL),
          moving q [128,4] -> scores[t, 4h] in PSUM
    p   = exp(scores*scale) (ACT, fp16 out) * mask (DVE fp16)
    PV:   per chunk: stationary V-chunk [128t,128d] (FWL), moving p-chunk
          [128,4] -> accumulates out [128d, 4h] in PSUM
    den:  ones[128,1]^T @ p -> per-(chunk,head) sums; host reduces + divides
- Outputs batched into 2 final DMAs; host does the final normalization.
"""
import numpy as np

B, H, KVH, D, X = 16, 16, 4, 128, 4
GRP = H // KVH              # q heads per kv head
BLK, MAXB = 16, 256
NC_CORES = 8
NSLOTS = (B * KVH) // NC_CORES   # panels per core
SM_SCALE = 1.0 / float(np.sqrt(D))

_PROG_CACHE = {}


def _build_device_program(slot_nch):
    import concourse.bacc as bacc
    import concourse.mybir as mybir
    from concourse.tile import TileContext

    f32 = mybir.dt.float32
    f16 = mybir.dt.float16
    nc = bacc.Bacc("TRN2", target_bir_lowering=False)
    # per-slot [K | mask] and [V] panels
    km = [nc.dram_tensor(f"km{s}", [128, slot_nch[s] * 132], f16, kind="ExternalInput")
          for s in range(NSLOTS)]
    vv = [nc.dram_tensor(f"vv{s}", [128, slot_nch[s] * 128], f16, kind="ExternalInput")
          for s in range(NSLOTS)]
    qd = nc.dram_tensor("qd", [128, 4 * NSLOTS], f16, kind="ExternalInput")
    oud = nc.dram_tensor("oud", [128, 4 * NSLOTS], f32, kind="ExternalOutput")
    dend = nc.dram_tensor("dend", [1, 4 * NSLOTS], f32, kind="ExternalOutput")

    with TileContext(nc) as tc:
        with (
            tc.tile_pool(name="data", bufs=1) as dp,
            tc.tile_pool(name="work", bufs=3) as wp,
            tc.tile_pool(name="ps_sc", bufs=2, space="PSUM") as psc,
            tc.tile_pool(name="ps_ov", bufs=2, space="PSUM") as pov,
            tc.tile_pool(name="ps_ds", bufs=2, space="PSUM") as pds,
        ):
            qt = dp.tile([128, 4 * NSLOTS], f16, tag="q")
            nc.sync.dma_start(out=qt[:], in_=qd[:])
            ones = dp.tile([128, 1], f16, tag="ones")
            nc.vector.memset(ones[:], 1.0)
            osb = dp.tile([128, 4 * NSLOTS], f32, tag="osb")
            dsb = dp.tile([1, 4 * NSLOTS], f32, tag="dsb")

            # K pieces as separate tiles: chunk boundaries per slot so QK(s)
            # can start on a piece while the rest of the slot still streams.
            def piece_cuts(s, nch):
                if s == 0:
                    cuts = [0, 4, (nch + 4) // 2, nch]
                elif s in (1, 2):
                    cuts = [0, (nch + 1) // 2, nch]
                else:
                    cuts = [0, nch]
                return sorted(set(c for c in cuts if 0 <= c <= nch))

            kp = [None] * NSLOTS   # list of (tile, c0, c1) pieces; mask in last
            vvs = [None] * NSLOTS

            def issue_km(s):
                nch = slot_nch[s]
                cuts = piece_cuts(s, nch)
                pieces = []
                for i, (c0, c1) in enumerate(zip(cuts[:-1], cuts[1:])):
                    last = i == len(cuts) - 2
                    w = (c1 - c0) * 128 + (nch * 4 if last else 0)
                    t = dp.tile([128, w], f16, tag=f"km{s}_{i}")
                    nc.sync.dma_start(
                        out=t[:], in_=km[s][:, c0 * 128:c0 * 128 + w])
                    pieces.append((t, c0, c1))
                kp[s] = pieces

            def issue_vv(s):
                nch = slot_nch[s]
                vvt = dp.tile([128, nch * 128], f16, tag=f"vv{s}")
                nc.sync.dma_start(out=vvt[:], in_=vv[s][:])
                vvs[s] = vvt

            # stream order: K runs one slot ahead of V so PE's QK is never
            # the last thing waiting on data
            issue_km(0)
            issue_km(1)
            issue_vv(0)
            for s in range(2, NSLOTS):
                issue_km(s)
                issue_vv(s - 1)
            issue_vv(NSLOTS - 1)

            ps = [None] * NSLOTS

            def qk_softmax(s):
                nch = slot_nch[s]
                sc = psc.tile([128, nch * 4], f32, tag="sc")
                for t, c0, c1 in kp[s]:
                    for c in range(c0, c1):
                        nc.tensor.matmul(
                            sc[:, 4 * c:4 * c + 4],
                            t[:, 128 * (c - c0):128 * (c - c0 + 1)],
                            qt[:, 4 * s:4 * s + 4],
                            start=True, stop=True,
                        )
                p0 = wp.tile([128, nch * 4], f16, tag="p0")
                nc.scalar.activation(
                    p0[:], sc[:], mybir.ActivationFunctionType.Exp,
                    scale=SM_SCALE,
                )
                lt, lc0, lc1 = kp[s][-1]
                moff = 128 * (lc1 - lc0)
                p = wp.tile([128, nch * 4], f16, tag="p")
                nc.vector.tensor_mul(
                    p[:], p0[:], lt[:, moff:moff + nch * 4])
                ps[s] = p

            def pv(s):
                nch = slot_nch[s]
                ov = pov.tile([128, 4], f32, tag="ov")
                for c in range(nch):
                    nc.tensor.matmul(
                        ov[:],
                        vvs[s][:, 128 * c:128 * (c + 1)],
                        ps[s][:, 4 * c:4 * c + 4],
                        start=(c == 0), stop=(c == nch - 1),
                    )
                ds = pds.tile([1, nch * 4], f32, tag="ds")
                nc.tensor.matmul(ds[:], ones[:], ps[s][:], start=True, stop=True)
                nc.vector.tensor_copy(osb[:, 4 * s:4 * s + 4], ov[:])
                nc.vector.tensor_reduce(
                    dsb[:, 4 * s:4 * s + 4],
                    ds[:].rearrange("p (c h) -> p h c", h=4),
                    mybir.AxisListType.X, mybir.AluOpType.add)
                ps[s] = None

            # software pipeline: PE does QK(s+1) while ACT/DVE produce p(s);
            # PE never waits on the softmax chain.
            qk_softmax(0)
            for s in range(1, NSLOTS):
                qk_softmax(s)
                pv(s - 1)
            pv(NSLOTS - 1)

            nc.sync.dma_start(out=oud[:], in_=osb[:])
            nc.scalar.dma_start(out=dend[:], in_=dsb[:])
    nc.compile()
    return nc


def _prep(q, k_cache, v_cache, block_tables, context_lens, layout_crow, layout_col):
    """Resolve CSR rows, dedup kv blocks per (b, kv-head), build panels."""
    q_pid = context_lens.astype(np.int64) - 1            # [B]
    pbid = q_pid // BLK
    h_idx = np.arange(H)
    start = layout_crow[h_idx[None, :], pbid[:, None]]   # [B,H]
    end = layout_crow[h_idx[None, :], pbid[:, None] + 1]

    panels = []  # (nch, b, kv, U, cols_per_head)
    for b in range(B):
        for kv in range(KVH):
            cols_h = []
            for dh in range(GRP):
                h = kv * GRP + dh
                cols_h.append(layout_col[h, start[b, h]:end[b, h]])
            U = np.unique(np.concatenate(cols_h))
            nch = max(1, -(-(len(U) * BLK) // 128))
            panels.append((nch, b, kv, U, cols_h))

    order = sorted(range(len(panels)), key=lambda i: -panels[i][0])
    # slot processing order: smallest group first (compute starts after a
    # small DMA), then descending, second-smallest last (short pipeline drain)
    group_to_pos = list(range(NSLOTS))   # size-rank group -> slot position (descending)
    slot_nch = [0] * NSLOTS
    assign = [[None] * NSLOTS for _ in range(NC_CORES)]
    for rank, pi in enumerate(order):
        core, g = rank % NC_CORES, rank // NC_CORES
        pos = group_to_pos[g]
        assign[core][pos] = pi
        if core == 0:
            slot_nch[pos] = panels[pi][0]
    slot_nch = tuple(slot_nch)

    in_maps = []
    meta = []    # per core: list of (b, kv) per slot
    tok16 = np.arange(BLK)
    for core in range(NC_CORES):
        im = {}
        mt_core = []
        qd = np.zeros((128, 4 * NSLOTS), np.float16)
        for s in range(NSLOTS):
            nch, b, kv, U, cols_h = panels[assign[core][s]]
            NT = slot_nch[s] * 128
            NU = len(U)
            phys = block_tables[b, U]

            kmt = np.zeros((128, slot_nch[s] * 132), np.float16)
            kb = k_cache[phys, kv]                       # [NU, 32, 16, 4]
            kmt[:, :NU * BLK] = kb.transpose(1, 3, 0, 2).reshape(128, NU * BLK)

            vb = v_cache[phys, kv]                       # [NU, 128, 16]
            v_t = np.zeros((NT, 128), np.float16)
            v_t[:NU * BLK] = vb.transpose(0, 2, 1).reshape(NU * BLK, 128)
            vvt = np.ascontiguousarray(
                v_t.reshape(slot_nch[s], 128, 128).transpose(1, 0, 2)
                .reshape(128, NT))

            mm = np.zeros((4, NT), np.float16)
            upos = U * BLK
            causal = (upos[:, None] + tok16[None, :]) <= q_pid[b]   # [NU,16]
            for dh in range(GRP):
                allowed = np.isin(U, cols_h[dh])[:, None] & causal
                mm[dh, :NU * BLK] = allowed.reshape(-1).astype(np.float16)
            kmt[:, NT:] = (
                mm.reshape(4, slot_nch[s], 128).transpose(2, 1, 0)
                .reshape(128, slot_nch[s] * 4))

            im[f"km{s}"] = kmt
            im[f"vv{s}"] = vvt
            qd[:, 4 * s:4 * s + 4] = q[b, kv * GRP:(kv + 1) * GRP].T
            mt_core.append((b, kv))
        im["qd"] = qd
        in_maps.append(im)
        meta.append(mt_core)
    return slot_nch, in_maps, meta


def kernel(q, k_cache, v_cache, block_tables, context_lens, layout_crow, layout_col):
    import os
    from concourse.bass_utils import run_bass_kernel_spmd

    q = np.asarray(q, np.float32)
    k_cache = np.asarray(k_cache, np.float32)
    v_cache = np.asarray(v_cache, np.float32)
    block_tables = np.asarray(block_tables, np.int32)
    context_lens = np.asarray(context_lens, np.int32)
    layout_crow = np.asarray(layout_crow, np.int32)
    layout_col = np.asarray(layout_col, np.int32)

    slot_nch, in_maps, meta = _prep(
        q, k_cache, v_cache, block_tables, context_lens, layout_crow, layout_col)

    nc = _PROG_CACHE.get(slot_nch)
    if nc is None:
        nc = _build_device_program(slot_nch)
        _PROG_CACHE[slot_nch] = nc

    res = run_bass_kernel_spmd(
        nc, in_maps, core_ids=list(range(NC_CORES)),
        trace=bool(os.environ.get("KERNEL_TRACE")),
    )
    global _LAST_RESULT
    _LAST_RESULT = res

    out = np.empty((B, H, D), np.float32)
    for core in range(NC_CORES):
        oud = res.results[core]["oud"]                   # [128, 4*NSLOTS]
        den = res.results[core]["dend"][0]               # [4*NSLOTS]
        for s in range(NSLOTS):
            b, kv = meta[core][s]
            out[b, kv * GRP:(kv + 1) * GRP] = (
                oud[:, 4 * s:4 * s + 4] / den[4 * s:4 * s + 4]).T
    return out


_LAST_RESULT = None


# revision 23
# speedup vs baseline: 1.9560x; 1.0783x over previous
"""Local-strided block-sparse paged attention (decode) on 8 Trainium2 cores.

Strategy (memory-bound -> minimize device HBM bytes and DMA/descriptor serialization):
- Host resolves the per-(batch, q-head) CSR rows, then DEDUPLICATES the kv
  blocks across the 4 q-heads of each kv-head group: one gathered K/V panel
  per (b, kv_head) union, with per-head multiplicative masks restoring each
  head's exact row + causal masking.
- Panels are fp16 (halves bytes; ~5e-4 rel err, tolerance 2e-2).
- The 64 panels are assigned to 8 cores x 8 slots sorted by size, so the
  SPMD slot padding (same program on all cores) wastes little bandwidth.
- Two DMAs per slot: [K|mask] issued by the Sync engine, [V] by the Scalar
  engine -- descriptor generation (~630ns/DMA) runs on both engines in
  parallel instead of serializing on Sync.
- Device per slot (all 4 heads batched per matmul):
    QK:   per 128-token chunk: stationary K-chunk [128d,128t] (FWL),
          moving q [128,4] -> scores[t, 4h] in PSUM
    p   = exp(scores*scale) (ACT, fp16 out) * mask (DVE fp16)
    PV:   per chunk: stationary V-chunk [128t,128d] (FWL), moving p-chunk
          [128,4] -> accumulates out [128d, 4h] in PSUM
    den:  ones[128,1]^T @ p -> per-(chunk,head) sums; host reduces + divides
- Outputs batched into 2 final DMAs; host does the final normalization.
"""
import numpy as np

B, H, KVH, D, X = 16, 16, 4, 128, 4
GRP = H // KVH              # q heads per kv head
BLK, MAXB = 16, 256
NC_CORES = 8
NSLOTS = (B * KVH) // NC_CORES   # panels per core
SM_SCALE = 1.0 / float(np.sqrt(D))

_PROG_CACHE = {}


def _build_device_program(slot_nch):
    import concourse.bacc as bacc
    import concourse.mybir as mybir
    from concourse.tile import TileContext

    f32 = mybir.dt.float32
    f16 = mybir.dt.float16
    nc = bacc.Bacc("TRN2", target_bir_lowering=False)
    # per-slot [K | mask] and [V] panels
    km = [nc.dram_tensor(f"km{s}", [128, slot_nch[s] * 132], f16, kind="ExternalInput")
          for s in range(NSLOTS)]
    vv = [nc.dram_tensor(f"vv{s}", [128, slot_nch[s] * 128], f16, kind="ExternalInput")
          for s in range(NSLOTS)]
    qd = nc.dram_tensor("qd", [128, 4 * NSLOTS], f16, kind="ExternalInput")
    oud = nc.dram_tensor("oud", [128, 4 * NSLOTS], f32, kind="ExternalOutput")
    dend = nc.dram_tensor("dend", [1, 4 * NSLOTS], f32, kind="ExternalOutput")

    with TileContext(nc) as tc:
        with (
            tc.tile_pool(name="data", bufs=1) as dp,
            tc.tile_pool(name="work", bufs=3) as wp,
            tc.tile_pool(name="ps_sc", bufs=2, space="PSUM") as psc,
            tc.tile_pool(name="ps_ov", bufs=2, space="PSUM") as pov,
            tc.tile_pool(name="ps_ds", bufs=2, space="PSUM") as pds,
        ):
            qt = dp.tile([128, 4 * NSLOTS], f16, tag="q")
            nc.sync.dma_start(out=qt[:], in_=qd[:])
            ones = dp.tile([128, 1], f16, tag="ones")
            nc.vector.memset(ones[:], 1.0)
            osb = dp.tile([128, 4 * NSLOTS], f32, tag="osb")
            dsb = dp.tile([1, 4 * NSLOTS], f32, tag="dsb")

            # K pieces as separate tiles: chunk boundaries per slot so QK(s)
            # can start on a piece while the rest of the slot still streams.
            def piece_cuts(s, nch):
                if s == 0:
                    cuts = [0, 4, (nch + 4) // 2, nch]
                elif s in (1, 2):
                    cuts = [0, (nch + 1) // 2, nch]
                else:
                    cuts = [0, nch]
                return sorted(set(c for c in cuts if 0 <= c <= nch))

            kp = [None] * NSLOTS   # list of (tile, c0, c1) pieces; mask in last
            vvs = [None] * NSLOTS

            def issue_km(s):
                nch = slot_nch[s]
                cuts = piece_cuts(s, nch)
                pieces = []
                for i, (c0, c1) in enumerate(zip(cuts[:-1], cuts[1:])):
                    last = i == len(cuts) - 2
                    w = (c1 - c0) * 128 + (nch * 4 if last else 0)
                    t = dp.tile([128, w], f16, tag=f"km{s}_{i}")
                    nc.sync.dma_start(
                        out=t[:], in_=km[s][:, c0 * 128:c0 * 128 + w])
                    pieces.append((t, c0, c1))
                kp[s] = pieces

            def issue_vv(s):
                nch = slot_nch[s]
                vvt = dp.tile([128, nch * 128], f16, tag=f"vv{s}")
                nc.scalar.dma_start(out=vvt[:], in_=vv[s][:])
                vvs[s] = vvt

            # K descriptors on Sync, V on Scalar: descriptor generation
            # (~0.6us per DMA) runs on both engines in parallel
            for s in range(NSLOTS):
                issue_km(s)
                issue_vv(s)

            ps = [None] * NSLOTS

            def qk_softmax(s):
                nch = slot_nch[s]
                sc = psc.tile([128, nch * 4], f32, tag="sc")
                for t, c0, c1 in kp[s]:
                    for c in range(c0, c1):
                        nc.tensor.matmul(
                            sc[:, 4 * c:4 * c + 4],
                            t[:, 128 * (c - c0):128 * (c - c0 + 1)],
                            qt[:, 4 * s:4 * s + 4],
                            start=True, stop=True,
                        )
                p0 = wp.tile([128, nch * 4], f16, tag="p0")
                nc.scalar.activation(
                    p0[:], sc[:], mybir.ActivationFunctionType.Exp,
                    scale=SM_SCALE,
                )
                lt, lc0, lc1 = kp[s][-1]
                moff = 128 * (lc1 - lc0)
                p = wp.tile([128, nch * 4], f16, tag="p")
                nc.vector.tensor_mul(
                    p[:], p0[:], lt[:, moff:moff + nch * 4])
                ps[s] = p

            def pv(s):
                nch = slot_nch[s]
                ov = pov.tile([128, 4], f32, tag="ov")
                for c in range(nch):
                    nc.tensor.matmul(
                        ov[:],
                        vvs[s][:, 128 * c:128 * (c + 1)],
                        ps[s][:, 4 * c:4 * c + 4],
                        start=(c == 0), stop=(c == nch - 1),
                    )
                ds = pds.tile([1, nch * 4], f32, tag="ds")
                nc.tensor.matmul(ds[:], ones[:], ps[s][:], start=True, stop=True)
                nc.vector.tensor_copy(osb[:, 4 * s:4 * s + 4], ov[:])
                nc.vector.tensor_reduce(
                    dsb[:, 4 * s:4 * s + 4],
                    ds[:].rearrange("p (c h) -> p h c", h=4),
                    mybir.AxisListType.X, mybir.AluOpType.add)
                ps[s] = None

            # software pipeline: PE does QK(s+1) while ACT/DVE produce p(s);
            # PE never waits on the softmax chain.
            qk_softmax(0)
            for s in range(1, NSLOTS):
                qk_softmax(s)
                pv(s - 1)
            pv(NSLOTS - 1)

            nc.sync.dma_start(out=oud[:], in_=osb[:])
            nc.scalar.dma_start(out=dend[:], in_=dsb[:])
    nc.compile()
    return nc


def _prep(q, k_cache, v_cache, block_tables, context_lens, layout_crow, layout_col):
    """Resolve CSR rows, dedup kv blocks per (b, kv-head), build panels."""
    q_pid = context_lens.astype(np.int64) - 1            # [B]
    pbid = q_pid // BLK
    h_idx = np.arange(H)
    start = layout_crow[h_idx[None, :], pbid[:, None]]   # [B,H]
    end = layout_crow[h_idx[None, :], pbid[:, None] + 1]

    panels = []  # (nch, b, kv, U, cols_per_head)
    for b in range(B):
        for kv in range(KVH):
            cols_h = []
            for dh in range(GRP):
                h = kv * GRP + dh
                cols_h.append(layout_col[h, start[b, h]:end[b, h]])
            U = np.unique(np.concatenate(cols_h))
            nch = max(1, -(-(len(U) * BLK) // 128))
            panels.append((nch, b, kv, U, cols_h))

    order = sorted(range(len(panels)), key=lambda i: -panels[i][0])
    # slot processing order: smallest group first (compute starts after a
    # small DMA), then descending, second-smallest last (short pipeline drain)
    group_to_pos = list(range(NSLOTS))   # size-rank group -> slot position (descending)
    slot_nch = [0] * NSLOTS
    assign = [[None] * NSLOTS for _ in range(NC_CORES)]
    for rank, pi in enumerate(order):
        core, g = rank % NC_CORES, rank // NC_CORES
        pos = group_to_pos[g]
        assign[core][pos] = pi
        if core == 0:
            slot_nch[pos] = panels[pi][0]
    slot_nch = tuple(slot_nch)

    in_maps = []
    meta = []    # per core: list of (b, kv) per slot
    tok16 = np.arange(BLK)
    for core in range(NC_CORES):
        im = {}
        mt_core = []
        qd = np.zeros((128, 4 * NSLOTS), np.float16)
        for s in range(NSLOTS):
            nch, b, kv, U, cols_h = panels[assign[core][s]]
            NT = slot_nch[s] * 128
            NU = len(U)
            phys = block_tables[b, U]

            kmt = np.zeros((128, slot_nch[s] * 132), np.float16)
            kb = k_cache[phys, kv]                       # [NU, 32, 16, 4]
            kmt[:, :NU * BLK] = kb.transpose(1, 3, 0, 2).reshape(128, NU * BLK)

            vb = v_cache[phys, kv]                       # [NU, 128, 16]
            v_t = np.zeros((NT, 128), np.float16)
            v_t[:NU * BLK] = vb.transpose(0, 2, 1).reshape(NU * BLK, 128)
            vvt = np.ascontiguousarray(
                v_t.reshape(slot_nch[s], 128, 128).transpose(1, 0, 2)
                .reshape(128, NT))

            mm = np.zeros((4, NT), np.float16)
            upos = U * BLK
            causal = (upos[:, None] + tok16[None, :]) <= q_pid[b]   # [NU,16]
            for dh in range(GRP):
                allowed = np.isin(U, cols_h[dh])[:, None] & causal
                mm[dh, :NU * BLK] = allowed.reshape(-1).astype(np.float16)
            kmt[:, NT:] = (
                mm.reshape(4, slot_nch[s], 128).transpose(2, 1, 0)
                .reshape(128, slot_nch[s] * 4))

            im[f"km{s}"] = kmt
            im[f"vv{s}"] = vvt
            qd[:, 4 * s:4 * s + 4] = q[b, kv * GRP:(kv + 1) * GRP].T
            mt_core.append((b, kv))
        im["qd"] = qd
        in_maps.append(im)
        meta.append(mt_core)
    return slot_nch, in_maps, meta


def kernel(q, k_cache, v_cache, block_tables, context_lens, layout_crow, layout_col):
    import os
    from concourse.bass_utils import run_bass_kernel_spmd

    q = np.asarray(q, np.float32)
    k_cache = np.asarray(k_cache, np.float32)
    v_cache = np.asarray(v_cache, np.float32)
    block_tables = np.asarray(block_tables, np.int32)
    context_lens = np.asarray(context_lens, np.int32)
    layout_crow = np.asarray(layout_crow, np.int32)
    layout_col = np.asarray(layout_col, np.int32)

    slot_nch, in_maps, meta = _prep(
        q, k_cache, v_cache, block_tables, context_lens, layout_crow, layout_col)

    nc = _PROG_CACHE.get(slot_nch)
    if nc is None:
        nc = _build_device_program(slot_nch)
        _PROG_CACHE[slot_nch] = nc

    res = run_bass_kernel_spmd(
        nc, in_maps, core_ids=list(range(NC_CORES)),
        trace=bool(os.environ.get("KERNEL_TRACE")),
    )
    global _LAST_RESULT
    _LAST_RESULT = res

    out = np.empty((B, H, D), np.float32)
    for core in range(NC_CORES):
        oud = res.results[core]["oud"]                   # [128, 4*NSLOTS]
        den = res.results[core]["dend"][0]               # [4*NSLOTS]
        for s in range(NSLOTS):
            b, kv = meta[core][s]
            out[b, kv * GRP:(kv + 1) * GRP] = (
                oud[:, 4 * s:4 * s + 4] / den[4 * s:4 * s + 4]).T
    return out


_LAST_RESULT = None
